# revision 10
# baseline (speedup 1.0000x reference)
"""Trainium2 Bass kernel for HFA loss (N=4096, C=1000, A=64) on 8 NeuronCores.

Math:
  d_nc = w_c - w_{t_n};  sigma2 = ratio * d CV_n d^T;  sigma1 = |w_c|^2 - |w_t|^2
  logits = y + 0.5*sigma2 + sigma1;  loss = mean CE(logits, t)

Per-sample-constant logit terms cancel in CE, so effective logits:
  L = y + |w_c|^2 + s*(q_nc - cross_nc),   s = 0.5*ratio
  q_nc    = vec(CV_n) . vec(w_c w_c^T)  -> tri-packed GEMM over K=2080
  cross_nc = (S_n w_{t_n}) . w_c        -> 64 more GEMM rows (S = CV + CV^T)
Device does one bf16 GEMM with K=2176 (=17*128; 2080 tri + 64 cross + 32 pad)
into PSUM, adds y' = y + |w|^2 (f32, host-folded), then a fused softmax-CE
epilogue. Each core reduces its 512 samples to one partial scalar; host sums.
"""

import sys

for _p in ("/opt/trn_rl_repo", "/opt/pypackages"):
    if _p not in sys.path:
        sys.path.append(_p)

import numpy as np
import ml_dtypes

import concourse.bass as bass
import concourse.mybir as mybir
import concourse.tile as tile
from concourse import bacc
from concourse.bass_utils import run_bass_kernel_spmd

N, C, A = 4096, 1000, 64
N_CORES = 8
NS = N // N_CORES            # 512 samples per core
M_TILES = NS // 128          # 4
K_TRI = A * (A + 1) // 2     # 2080
K_PAD = 2176                 # 17 * 128
K_TILES = K_PAD // 128       # 17
CSPLIT = 512                 # PSUM bank boundary inside the 1000-col output

BF16 = mybir.dt.bfloat16
F32 = mybir.dt.float32
AF = mybir.ActivationFunctionType
ALU = mybir.AluOpType

_cache = {}


def _build_program():
    nc = bacc.Bacc("TRN2")

    dt_in = nc.dram_tensor("dt", [M_TILES, 128, K_TILES, 128], BF16,
                           kind="ExternalInput")
    w2_in = nc.dram_tensor("w2kc", [K_TILES, 128, C], BF16,
                           kind="ExternalInput")
    yp_in = nc.dram_tensor("yp", [M_TILES, 128, C], F32, kind="ExternalInput")
    tgt_in = nc.dram_tensor("tgt", [128, M_TILES], F32,
                            kind="ExternalInput")
    out_d = nc.dram_tensor("partial", [1, 1], F32, kind="ExternalOutput")

    with tile.TileContext(nc) as tc:
        with (
            tc.tile_pool(name="singles", bufs=1) as singles,
            tc.tile_pool(name="dpool", bufs=8) as dpool,
            tc.tile_pool(name="ypool", bufs=2) as ypool,
            tc.tile_pool(name="lpool", bufs=2) as lpool,
            tc.tile_pool(name="spool", bufs=8) as spool,
            tc.tile_pool(name="ppool", bufs=2, space=bass.MemorySpace.PSUM) as ppool,
            tc.tile_pool(name="psum1", bufs=1, space=bass.MemorySpace.PSUM) as psum1,
        ):
            # stationary-shared rhs: 17 k-slabs of W2ext, each [128, 1000] bf16
            w2sb = []
            for kt in range(K_TILES):
                t = singles.tile([128, C], BF16, tag=f"w2_{kt}")
                nc.sync.dma_start(out=t[:], in_=w2_in[kt])
                w2sb.append(t)

            # iota row 0..999 as f32 (for the target-logit select)
            iota_i = singles.tile([128, C], mybir.dt.int32, tag="iota_i")
            nc.gpsimd.iota(iota_i[:], pattern=[[1, C]], base=0,
                           channel_multiplier=0)
            iota_f = singles.tile([128, C], F32, tag="iota_f")
            nc.vector.tensor_copy(out=iota_f[:], in_=iota_i[:])

            # 1/N ones column for the final cross-partition loss reduction
            ones = singles.tile([128, 1], F32, tag="ones")
            nc.vector.memset(ones[:], 1.0 / N)

            nll = singles.tile([128, M_TILES], F32, tag="nll")
            tgt_sb = singles.tile([128, M_TILES], F32, tag="tgt")
            nc.sync.dma_start(out=tgt_sb[:], in_=tgt_in[:])

            for m in range(M_TILES):
                acc = ppool.tile([128, C], F32, tag="acc")
                dm = dpool.tile([128, K_TILES, 128], BF16, tag="d")
                nc.sync.dma_start(out=dm[:], in_=dt_in[m])
                for kt in range(K_TILES):
                    nc.tensor.matmul(acc[:, :CSPLIT], lhsT=dm[:, kt, :],
                                     rhs=w2sb[kt][:, :CSPLIT],
                                     start=(kt == 0), stop=(kt == K_TILES - 1))
                    nc.tensor.matmul(acc[:, CSPLIT:], lhsT=dm[:, kt, :],
                                     rhs=w2sb[kt][:, CSPLIT:],
                                     start=(kt == 0), stop=(kt == K_TILES - 1))

                yt = ypool.tile([128, C], F32, tag="y")
                nc.sync.dma_start(out=yt[:], in_=yp_in[m])
                tg = tgt_sb[:, m:m + 1]

                # L = acc + y', then -rowmax(L)
                L = lpool.tile([128, C], F32, tag="L")
                nc.vector.tensor_add(L[:], acc[:], yt[:])
                mx = spool.tile([128, 1], F32, tag="mx")
                nc.vector.reduce_max(out=mx[:], in_=L[:],
                                     axis=mybir.AxisListType.X)
                nmx = spool.tile([128, 1], F32, tag="nmx")
                nc.vector.tensor_scalar_mul(nmx[:], mx[:], -1.0)

                # e = exp(L - mx), se = rowsum(e)  (scalar engine, fused accum)
                e = lpool.tile([128, C], F32, tag="e")
                se = spool.tile([128, 1], F32, tag="se")
                nc.scalar.activation(out=e[:], in_=L[:], func=AF.Exp,
                                     bias=nmx[:], scale=1.0, accum_out=se[:])

                # tl = L[p, tgt[p]] via (iota == tgt) * L, row-summed
                msk = lpool.tile([128, C], F32, tag="msk")
                tl = spool.tile([128, 1], F32, tag="tl")
                nc.vector.scalar_tensor_tensor(
                    out=msk[:], in0=iota_f[:], scalar=tg, in1=L[:],
                    op0=ALU.is_equal, op1=ALU.mult, accum_out=tl[:])

                # nll = ln(se) + mx - tl
                lnse = spool.tile([128, 1], F32, tag="lnse")
                nc.scalar.activation(out=lnse[:], in_=se[:], func=AF.Ln)
                nc.vector.scalar_tensor_tensor(
                    out=nll[:, m:m + 1], in0=lnse[:], scalar=mx[:], in1=tl[:],
                    op0=ALU.add, op1=ALU.subtract)

            # partial = sum_p sum_m nll / N
            ps = psum1.tile([1, M_TILES], F32, tag="ps")
            nc.tensor.matmul(ps[:], lhsT=ones[:], rhs=nll[:],
                             start=True, stop=True)
            res = spool.tile([1, 1], F32, tag="res")
            nc.vector.reduce_sum(out=res[:], in_=ps[:], axis=mybir.AxisListType.X)
            nc.sync.dma_start(out=out_d[:], in_=res[:])

    nc.finalize()
    return nc


def _host_prep(weight_m, y, target_x, ratio, final_conv):
    bf = ml_dtypes.bfloat16
    W = np.asarray(weight_m, dtype=np.float32)            # (C, A)
    y = np.ascontiguousarray(np.asarray(y, dtype=np.float32))
    tgt = np.asarray(target_x).astype(np.int64)
    CV = np.asarray(final_conv, dtype=np.float32)         # (N, A, A)
    s = 0.5 * float(np.asarray(ratio))

    iu0, iu1 = np.triu_indices(A)
    diag = iu0 == iu1

    # rhs: K_PAD x C bf16 (tri block, cross block, zero pad), scale folded in
    w2tri = W[:, iu0] * W[:, iu1]                         # (C, 2080)
    w2tri[:, diag] *= 0.5
    w2ext = np.zeros((K_PAD, C), dtype=bf)
    w2ext[:K_TRI] = (s * w2tri).T.astype(bf)
    w2ext[K_TRI:K_TRI + A] = (-s * W).T.astype(bf)
    w2kc = np.ascontiguousarray(w2ext.reshape(K_TILES, 128, C))

    # data: D = tri-packed (CV + CV^T), G = (CV + CV^T) @ w_t
    S = CV + CV.swapaxes(1, 2)                            # (N, A, A)
    dext = np.zeros((N, K_PAD), dtype=bf)
    dext[:, :K_TRI] = S[:, iu0, iu1].astype(bf)
    G = np.einsum('nab,na->nb', S, W[tgt], optimize=True)
    dext[:, K_TRI:K_TRI + A] = G.astype(bf)

    # y' = y + |w_c|^2  (f32, exact add on host)
    w2n = np.einsum('ca,ca->c', W, W)
    yp = y + w2n[None, :]

    in_maps = []
    for i in range(N_CORES):
        sl = slice(i * NS, (i + 1) * NS)
        dcore = dext[sl]                                  # (512, 2176)
        dt_i = np.ascontiguousarray(
            dcore.reshape(M_TILES, 128, K_TILES, 128).transpose(0, 3, 2, 1))
        yp_i = np.ascontiguousarray(yp[sl].reshape(M_TILES, 128, C))
        tg_i = np.ascontiguousarray(
            tgt[sl].astype(np.float32).reshape(M_TILES, 128).T)
        in_maps.append({"dt": dt_i, "w2kc": w2kc, "yp": yp_i, "tgt": tg_i})
    return in_maps


def kernel(weight_m, y, features, target_x, ratio, final_conv, class_num,
           _trace=False):
    if "nc" not in _cache:
        _cache["nc"] = _build_program()
    nc = _cache["nc"]

    in_maps = _host_prep(weight_m, y, target_x, ratio, final_conv)
    r = run_bass_kernel_spmd(nc, in_maps, core_ids=list(range(N_CORES)),
                             trace=_trace)
    loss = np.float32(sum(float(res["partial"][0, 0]) for res in r.results))
    y_out = np.asarray(y, dtype=np.float32)
    if _trace:
        _cache["last_results"] = r
    return loss, y_out


# revision 12
# speedup vs baseline: 1.1030x; 1.1030x over previous
"""Trainium2 Bass kernel for HFA loss (N=4096, C=1000, A=64) on 8 NeuronCores.

Math:
  d_nc = w_c - w_{t_n};  sigma2 = ratio * d CV_n d^T;  sigma1 = |w_c|^2 - |w_t|^2
  logits = y + 0.5*sigma2 + sigma1;  loss = mean CE(logits, t)

Per-sample-constant logit terms cancel in CE, so effective logits:
  L = y + |w_c|^2 + s*(q_nc - cross_nc),   s = 0.5*ratio
  q_nc    = vec(CV_n) . vec(w_c w_c^T)  -> tri-packed GEMM over K=2080
  cross_nc = (S_n w_{t_n}) . w_c        -> 64 more GEMM rows (S = CV + CV^T)
Device does one bf16 GEMM with K=2176 (=17*128; 2080 tri + 64 cross + 32 pad)
into PSUM, adds y' = y + |w|^2 (f32, host-folded), then a fused softmax-CE
epilogue. Each core reduces its 512 samples to one partial scalar; host sums.
"""

import sys

for _p in ("/opt/trn_rl_repo", "/opt/pypackages"):
    if _p not in sys.path:
        sys.path.append(_p)

import numpy as np
import ml_dtypes

import concourse.bass as bass
import concourse.mybir as mybir
import concourse.tile as tile
from concourse import bacc
from concourse.bass_utils import run_bass_kernel_spmd

N, C, A = 4096, 1000, 64
N_CORES = 8
NS = N // N_CORES            # 512 samples per core
M_TILES = NS // 128          # 4
K_TRI = A * (A + 1) // 2     # 2080
K_PAD = 2176                 # 17 * 128
K_TILES = K_PAD // 128       # 17
CSPLIT = 512                 # PSUM bank boundary inside the 1000-col output

BF16 = mybir.dt.bfloat16
F32 = mybir.dt.float32
AF = mybir.ActivationFunctionType
ALU = mybir.AluOpType

_cache = {}


def _build_program():
    nc = bacc.Bacc("TRN2")

    dt_in = nc.dram_tensor("dt", [M_TILES, 128, K_TILES, 128], BF16,
                           kind="ExternalInput")
    w2_in = nc.dram_tensor("w2kc", [K_TILES, 128, C], BF16,
                           kind="ExternalInput")
    yp_in = nc.dram_tensor("yp", [M_TILES, 128, C], F32, kind="ExternalInput")
    tgt_in = nc.dram_tensor("tgt", [128, M_TILES], F32,
                            kind="ExternalInput")
    out_d = nc.dram_tensor("partial", [1, 1], F32, kind="ExternalOutput")

    with tile.TileContext(nc) as tc:
        with (
            tc.tile_pool(name="singles", bufs=1) as singles,
            tc.tile_pool(name="dpool", bufs=3) as dpool,
            tc.tile_pool(name="ypool", bufs=2) as ypool,
            tc.tile_pool(name="lpool", bufs=2) as lpool,
            tc.tile_pool(name="spool", bufs=8) as spool,
            tc.tile_pool(name="ppool", bufs=2, space=bass.MemorySpace.PSUM) as ppool,
            tc.tile_pool(name="psum1", bufs=1, space=bass.MemorySpace.PSUM) as psum1,
        ):
            # data tile for m=0 first so the GEMM can start as soon as the
            # first W2 slab lands (W2 is 4.4MB; serializing it first would
            # starve the PE for ~13us)
            dm_tiles = [dpool.tile([128, K_TILES, 128], BF16, tag=f"d{m}",
                                   name=f"d{m}")
                        for m in range(M_TILES)]
            nc.sync.dma_start(out=dm_tiles[0][:], in_=dt_in[0])

            # stationary-shared rhs: 17 k-slabs of W2ext, each [128, 1000] bf16
            w2sb = []
            for kt in range(K_TILES):
                t = singles.tile([128, C], BF16, tag=f"w2_{kt}")
                nc.sync.dma_start(out=t[:], in_=w2_in[kt])
                w2sb.append(t)
            for m in range(1, M_TILES):
                nc.sync.dma_start(out=dm_tiles[m][:], in_=dt_in[m])

            # iota row 0..999 as f32 (for the target-logit select)
            iota_i = singles.tile([128, C], mybir.dt.int32, tag="iota_i")
            nc.gpsimd.iota(iota_i[:], pattern=[[1, C]], base=0,
                           channel_multiplier=0)
            iota_f = singles.tile([128, C], F32, tag="iota_f")
            nc.vector.tensor_copy(out=iota_f[:], in_=iota_i[:])

            # 1/N ones column for the final cross-partition loss reduction
            ones = singles.tile([128, 1], F32, tag="ones")
            nc.vector.memset(ones[:], 1.0 / N)

            tgt_sb = singles.tile([128, M_TILES], F32, tag="tgt")
            nc.sync.dma_start(out=tgt_sb[:], in_=tgt_in[:])

            se_all = singles.tile([128, M_TILES], F32, tag="se_all")
            mx_all = singles.tile([128, M_TILES], F32, tag="mx_all")
            tl_all = singles.tile([128, M_TILES], F32, tag="tl_all")

            for m in range(M_TILES):
                acc = ppool.tile([128, C], F32, tag="acc")
                dm = dm_tiles[m]
                for kt in range(K_TILES):
                    nc.tensor.matmul(acc[:, :CSPLIT], lhsT=dm[:, kt, :],
                                     rhs=w2sb[kt][:, :CSPLIT],
                                     start=(kt == 0), stop=(kt == K_TILES - 1))
                    nc.tensor.matmul(acc[:, CSPLIT:], lhsT=dm[:, kt, :],
                                     rhs=w2sb[kt][:, CSPLIT:],
                                     start=(kt == 0), stop=(kt == K_TILES - 1))

                yt = ypool.tile([128, C], F32, tag="y")
                nc.sync.dma_start(out=yt[:], in_=yp_in[m])

                # L = acc + y', then -rowmax(L)
                L = lpool.tile([128, C], F32, tag="L")
                nc.vector.tensor_add(L[:], acc[:], yt[:])
                mx = mx_all[:, m:m + 1]
                nc.vector.reduce_max(out=mx, in_=L[:],
                                     axis=mybir.AxisListType.X)
                nmx = spool.tile([128, 1], F32, tag="nmx")
                nc.vector.tensor_scalar_mul(nmx[:], mx, -1.0)

                # e = exp(L - mx), se = rowsum(e)  (scalar engine, fused accum)
                e = lpool.tile([128, C], F32, tag="e")
                nc.scalar.activation(out=e[:], in_=L[:], func=AF.Exp,
                                     bias=nmx[:], scale=1.0,
                                     accum_out=se_all[:, m:m + 1])

                # tl = L[p, tgt[p]] via (iota == tgt) * L, row-summed
                msk = lpool.tile([128, C], F32, tag="msk")
                nc.vector.scalar_tensor_tensor(
                    out=msk[:], in0=iota_f[:], scalar=tgt_sb[:, m:m + 1],
                    in1=L[:], op0=ALU.is_equal, op1=ALU.mult,
                    accum_out=tl_all[:, m:m + 1])

            # nll = ln(se) + mx - tl, all m at once (one act-table swap)
            lnse = singles.tile([128, M_TILES], F32, tag="lnse")
            nc.scalar.activation(out=lnse[:], in_=se_all[:], func=AF.Ln)
            nll = singles.tile([128, M_TILES], F32, tag="nll")
            nc.vector.tensor_sub(nll[:], lnse[:], tl_all[:])
            nc.vector.tensor_add(nll[:], nll[:], mx_all[:])

            # partial = sum_p sum_m nll / N
            ps = psum1.tile([1, M_TILES], F32, tag="ps")
            nc.tensor.matmul(ps[:], lhsT=ones[:], rhs=nll[:],
                             start=True, stop=True)
            res = spool.tile([1, 1], F32, tag="res")
            nc.vector.reduce_sum(out=res[:], in_=ps[:], axis=mybir.AxisListType.X)
            nc.sync.dma_start(out=out_d[:], in_=res[:])

    nc.finalize()
    return nc


def _host_prep(weight_m, y, target_x, ratio, final_conv):
    bf = ml_dtypes.bfloat16
    W = np.asarray(weight_m, dtype=np.float32)            # (C, A)
    y = np.ascontiguousarray(np.asarray(y, dtype=np.float32))
    tgt = np.asarray(target_x).astype(np.int64)
    CV = np.asarray(final_conv, dtype=np.float32)         # (N, A, A)
    s = 0.5 * float(np.asarray(ratio))

    iu0, iu1 = np.triu_indices(A)
    diag = iu0 == iu1

    # rhs: K_PAD x C bf16 (tri block, cross block, zero pad), scale folded in
    w2tri = W[:, iu0] * W[:, iu1]                         # (C, 2080)
    w2tri[:, diag] *= 0.5
    w2ext = np.zeros((K_PAD, C), dtype=bf)
    w2ext[:K_TRI] = (s * w2tri).T.astype(bf)
    w2ext[K_TRI:K_TRI + A] = (-s * W).T.astype(bf)
    w2kc = np.ascontiguousarray(w2ext.reshape(K_TILES, 128, C))

    # data: D = tri-packed (CV + CV^T), G = (CV + CV^T) @ w_t
    S = CV + CV.swapaxes(1, 2)                            # (N, A, A)
    dext = np.zeros((N, K_PAD), dtype=bf)
    dext[:, :K_TRI] = S[:, iu0, iu1].astype(bf)
    G = np.einsum('nab,na->nb', S, W[tgt], optimize=True)
    dext[:, K_TRI:K_TRI + A] = G.astype(bf)

    # y' = y + |w_c|^2  (f32, exact add on host)
    w2n = np.einsum('ca,ca->c', W, W)
    yp = y + w2n[None, :]

    in_maps = []
    for i in range(N_CORES):
        sl = slice(i * NS, (i + 1) * NS)
        dcore = dext[sl]                                  # (512, 2176)
        dt_i = np.ascontiguousarray(
            dcore.reshape(M_TILES, 128, K_TILES, 128).transpose(0, 3, 2, 1))
        yp_i = np.ascontiguousarray(yp[sl].reshape(M_TILES, 128, C))
        tg_i = np.ascontiguousarray(
            tgt[sl].astype(np.float32).reshape(M_TILES, 128).T)
        in_maps.append({"dt": dt_i, "w2kc": w2kc, "yp": yp_i, "tgt": tg_i})
    return in_maps


def kernel(weight_m, y, features, target_x, ratio, final_conv, class_num,
           _trace=False):
    if "nc" not in _cache:
        _cache["nc"] = _build_program()
    nc = _cache["nc"]

    in_maps = _host_prep(weight_m, y, target_x, ratio, final_conv)
    r = run_bass_kernel_spmd(nc, in_maps, core_ids=list(range(N_CORES)),
                             trace=_trace)
    loss = np.float32(sum(float(res["partial"][0, 0]) for res in r.results))
    y_out = np.asarray(y, dtype=np.float32)
    if _trace:
        _cache["last_results"] = r
    return loss, y_out


# revision 19
# speedup vs baseline: 1.1176x; 1.0132x over previous
"""Trainium2 Bass kernel for HFA loss (N=4096, C=1000, A=64) on 8 NeuronCores.

Math:
  d_nc = w_c - w_{t_n};  sigma2 = ratio * d CV_n d^T;  sigma1 = |w_c|^2 - |w_t|^2
  logits = y + 0.5*sigma2 + sigma1;  loss = mean CE(logits, t)

Per-sample-constant logit terms cancel in CE, so effective logits:
  L = y + |w_c|^2 + s*(q_nc - cross_nc),   s = 0.5*ratio
  q_nc    = vec(CV_n) . vec(w_c w_c^T)  -> tri-packed GEMM over K=2080
  cross_nc = (S_n w_{t_n}) . w_c        -> 64 more GEMM rows (S = CV + CV^T)
Device does one bf16 GEMM with K=2176 (=17*128; 2080 tri + 64 cross + 32 pad)
into PSUM, adds y' = y + |w|^2 (f32, host-folded), then a fused softmax-CE
epilogue. Each core reduces its 512 samples to one partial scalar; host sums.
"""

import sys

for _p in ("/opt/trn_rl_repo", "/opt/pypackages"):
    if _p not in sys.path:
        sys.path.append(_p)

import numpy as np
import ml_dtypes

import concourse.bass as bass
import concourse.mybir as mybir
import concourse.tile as tile
from concourse import bacc
from concourse.bass_utils import run_bass_kernel_spmd

N, C, A = 4096, 1000, 64
N_CORES = 8
NS = N // N_CORES            # 512 samples per core
M_TILES = NS // 128          # 4
K_TRI = A * (A + 1) // 2     # 2080
K_PAD = 2176                 # 17 * 128
K_TILES = K_PAD // 128       # 17
CSPLIT = 512                 # PSUM bank boundary inside the 1000-col output

BF16 = mybir.dt.bfloat16
F32 = mybir.dt.float32
AF = mybir.ActivationFunctionType
ALU = mybir.AluOpType

_cache = {}


def _build_program():
    nc = bacc.Bacc("TRN2")

    dt_in = nc.dram_tensor("dt", [M_TILES, 128, K_TILES, 128], BF16,
                           kind="ExternalInput")
    w2_in = nc.dram_tensor("w2kc", [K_TILES, 128, C], BF16,
                           kind="ExternalInput")
    yp_in = nc.dram_tensor("yp", [M_TILES, 128, C], F32, kind="ExternalInput")
    tgt_in = nc.dram_tensor("tgt", [128, M_TILES], F32,
                            kind="ExternalInput")
    out_d = nc.dram_tensor("partial", [1, 1], F32, kind="ExternalOutput")

    with tile.TileContext(nc) as tc:
        with (
            tc.tile_pool(name="singles", bufs=1) as singles,
            tc.tile_pool(name="dpool", bufs=1) as dpool,
            tc.tile_pool(name="ypool", bufs=2) as ypool,
            tc.tile_pool(name="lpool", bufs=2) as lpool,
            tc.tile_pool(name="spool", bufs=8) as spool,
            tc.tile_pool(name="ppool", bufs=3, space=bass.MemorySpace.PSUM) as ppool,
            tc.tile_pool(name="psum1", bufs=1, space=bass.MemorySpace.PSUM) as psum1,
        ):
            # data tiles for m=0 first so the GEMM can start as soon as the
            # first W2 slab lands (W2 is 4.4MB; serializing it first would
            # starve the PE for ~13us). m=0 is further split so its first
            # two k-slabs (64KB) land well before the remaining 480KB.
            dm_tiles = [dpool.tile([128, K_TILES, 128], BF16, tag=f"d{m}",
                                   name=f"d{m}")
                        for m in range(1, M_TILES)]
            dm0a = dpool.tile([128, 2, 128], BF16, tag="d0a")
            dm0b = dpool.tile([128, K_TILES - 2, 128], BF16, tag="d0b")
            nc.sync.dma_start(out=dm0a[:], in_=dt_in[0, :, 0:2, :])

            # stationary-shared rhs: 17 k-slabs of W2ext, each [128, 1000] bf16
            w2sb = []
            for kt in range(K_TILES):
                t = singles.tile([128, C], BF16, tag=f"w2_{kt}")
                nc.sync.dma_start(out=t[:], in_=w2_in[kt])
                w2sb.append(t)
                if kt == 1:
                    nc.sync.dma_start(out=dm0b[:], in_=dt_in[0, :, 2:, :])
            for m in range(1, M_TILES):
                nc.sync.dma_start(out=dm_tiles[m - 1][:], in_=dt_in[m])

            # iota row 0..999 as f32 (for the target-logit select)
            iota_i = singles.tile([128, C], mybir.dt.int32, tag="iota_i")
            nc.gpsimd.iota(iota_i[:], pattern=[[1, C]], base=0,
                           channel_multiplier=0)
            iota_f = singles.tile([128, C], F32, tag="iota_f")
            nc.vector.tensor_copy(out=iota_f[:], in_=iota_i[:])

            # 1/N ones column for the final cross-partition loss reduction
            ones = singles.tile([128, 1], F32, tag="ones")
            nc.vector.memset(ones[:], 1.0 / N)

            tgt_sb = singles.tile([128, M_TILES], F32, tag="tgt")
            nc.sync.dma_start(out=tgt_sb[:], in_=tgt_in[:])

            se_all = singles.tile([128, M_TILES], F32, tag="se_all")
            mx_all = singles.tile([128, M_TILES], F32, tag="mx_all")
            tl_all = singles.tile([128, M_TILES], F32, tag="tl_all")

            for m in range(M_TILES):
                acc = ppool.tile([128, C], F32, tag="acc")
                for kt in range(K_TILES):
                    if m == 0:
                        lhsT = dm0a[:, kt, :] if kt < 2 else dm0b[:, kt - 2, :]
                    else:
                        lhsT = dm_tiles[m - 1][:, kt, :]
                    nc.tensor.matmul(acc[:, :CSPLIT], lhsT=lhsT,
                                     rhs=w2sb[kt][:, :CSPLIT],
                                     start=(kt == 0), stop=(kt == K_TILES - 1))
                    nc.tensor.matmul(acc[:, CSPLIT:], lhsT=lhsT,
                                     rhs=w2sb[kt][:, CSPLIT:],
                                     start=(kt == 0), stop=(kt == K_TILES - 1))

                yt = ypool.tile([128, C], F32, tag="y")
                nc.sync.dma_start(out=yt[:], in_=yp_in[m])

                # L = acc + y', then -rowmax(L) (negate fused into the reduce)
                L = lpool.tile([128, C], F32, tag="L")
                nc.vector.tensor_add(L[:], acc[:], yt[:])
                nmx = mx_all[:, m:m + 1]
                nc.vector.reduce_max(out=nmx, in_=L[:],
                                     axis=mybir.AxisListType.X, negate=True)

                # e = exp(L - mx), se = rowsum(e)  (scalar engine, fused accum)
                e = lpool.tile([128, C], F32, tag="e")
                nc.scalar.activation(out=e[:], in_=L[:], func=AF.Exp,
                                     bias=nmx, scale=1.0,
                                     accum_out=se_all[:, m:m + 1])

                # tl = L[p, tgt[p]] via (iota == tgt) * L, row-summed
                msk = lpool.tile([128, C], F32, tag="msk")
                nc.vector.scalar_tensor_tensor(
                    out=msk[:], in0=iota_f[:], scalar=tgt_sb[:, m:m + 1],
                    in1=L[:], op0=ALU.is_equal, op1=ALU.mult,
                    accum_out=tl_all[:, m:m + 1])

            # nll = ln(se) - nmx - tl, all m at once (one act-table swap);
            # mx_all holds the NEGATED maxima
            lnse = singles.tile([128, M_TILES], F32, tag="lnse")
            nc.scalar.activation(out=lnse[:], in_=se_all[:], func=AF.Ln)
            nll = singles.tile([128, M_TILES], F32, tag="nll")
            nc.vector.tensor_sub(nll[:], lnse[:], tl_all[:])
            nc.vector.tensor_sub(nll[:], nll[:], mx_all[:])

            # partial = sum_p sum_m nll / N
            ps = psum1.tile([1, M_TILES], F32, tag="ps")
            nc.tensor.matmul(ps[:], lhsT=ones[:], rhs=nll[:],
                             start=True, stop=True)
            res = spool.tile([1, 1], F32, tag="res")
            nc.vector.reduce_sum(out=res[:], in_=ps[:], axis=mybir.AxisListType.X)
            nc.sync.dma_start(out=out_d[:], in_=res[:])

    nc.finalize()
    return nc


def _host_prep(weight_m, y, target_x, ratio, final_conv):
    bf = ml_dtypes.bfloat16
    W = np.asarray(weight_m, dtype=np.float32)            # (C, A)
    y = np.ascontiguousarray(np.asarray(y, dtype=np.float32))
    tgt = np.asarray(target_x).astype(np.int64)
    CV = np.asarray(final_conv, dtype=np.float32)         # (N, A, A)
    s = 0.5 * float(np.asarray(ratio))

    iu0, iu1 = np.triu_indices(A)
    diag = iu0 == iu1

    # rhs: K_PAD x C bf16 (tri block, cross block, zero pad), scale folded in
    w2tri = W[:, iu0] * W[:, iu1]                         # (C, 2080)
    w2tri[:, diag] *= 0.5
    w2ext = np.zeros((K_PAD, C), dtype=bf)
    w2ext[:K_TRI] = (s * w2tri).T.astype(bf)
    w2ext[K_TRI:K_TRI + A] = (-s * W).T.astype(bf)
    w2kc = np.ascontiguousarray(w2ext.reshape(K_TILES, 128, C))

    # data: D = tri-packed (CV + CV^T), G = (CV + CV^T) @ w_t
    S = CV + CV.swapaxes(1, 2)                            # (N, A, A)
    dext = np.zeros((N, K_PAD), dtype=bf)
    dext[:, :K_TRI] = S[:, iu0, iu1].astype(bf)
    G = np.einsum('nab,na->nb', S, W[tgt], optimize=True)
    dext[:, K_TRI:K_TRI + A] = G.astype(bf)

    # y' = y + |w_c|^2  (f32, exact add on host)
    w2n = np.einsum('ca,ca->c', W, W)
    yp = y + w2n[None, :]

    in_maps = []
    for i in range(N_CORES):
        sl = slice(i * NS, (i + 1) * NS)
        dcore = dext[sl]                                  # (512, 2176)
        dt_i = np.ascontiguousarray(
            dcore.reshape(M_TILES, 128, K_TILES, 128).transpose(0, 3, 2, 1))
        yp_i = np.ascontiguousarray(yp[sl].reshape(M_TILES, 128, C))
        tg_i = np.ascontiguousarray(
            tgt[sl].astype(np.float32).reshape(M_TILES, 128).T)
        in_maps.append({"dt": dt_i, "w2kc": w2kc, "yp": yp_i, "tgt": tg_i})
    return in_maps


def kernel(weight_m, y, features, target_x, ratio, final_conv, class_num,
           _trace=False):
    if "nc" not in _cache:
        _cache["nc"] = _build_program()
    nc = _cache["nc"]

    in_maps = _host_prep(weight_m, y, target_x, ratio, final_conv)
    r = run_bass_kernel_spmd(nc, in_maps, core_ids=list(range(N_CORES)),
                             trace=_trace)
    loss = np.float32(sum(float(res["partial"][0, 0]) for res in r.results))
    y_out = np.asarray(y, dtype=np.float32)
    if _trace:
        _cache["last_results"] = r
    return loss, y_out
